# revision 1
# baseline (speedup 1.0000x reference)
"""Grouped MoE MLP (SwiGLU) for Trainium2, expert-parallel across 8 NeuronCores.

Problem: out = gmm(silu(gmm(x,Wg)) * gmm(x,Wu), Wd) with E=8 experts,
T=8192 tokens pre-sorted by expert, H=2048, I=4096.

Strategy: expert parallelism — core e computes expert e's tokens end-to-end.
The host splits the (ragged) token dim by expert, pads each group to a fixed
capacity C, and casts to bf16.  Weights ship in their NATIVE layouts
([H,I] for gate/up, [I,H] for down) — the device gathers them into SBUF
with rearranged-AP DMAs:
  * gate/up are consumed in PAIRS of 128-wide I-blocks, so each DMA line is
    256 contiguous columns = 512 B (line-rate), one DMA per pair per weight.
  * down is loaded per 512-wide H-block as one DMA of 1 KB lines.

Device program per core (all shapes hardcoded at build time):
  GEMM1 computes the SwiGLU intermediate TRANSPOSED (interT[I, C]) so that
  GEMM2's contraction dim (I) is already the partition dim — no on-device
  transposes anywhere. bf16 inputs, fp32 PSUM accumulation, bf16 output.
  The wg stream issues on the sync HWDGE ring and wu on the scalar ring so
  the ~0.6us per-DMA issue cost is split across both rings.
"""

import numpy as np
import ml_dtypes

P = 128          # partition dim
NB = 512         # matmul moving free dim / PSUM bank width (fp32)
E, T, H, I = 8, 8192, 2048, 4096
C_DEFAULT = T // E  # per-expert token capacity

_NC_CACHE = {}


def _build(C, Hd, Id, nb=NB):
    """Build + bacc-compile the per-core Tile program. Returns the Bass module."""
    import concourse.bass as bass  # noqa: F401
    import concourse.tile as tile
    from concourse import bacc, mybir

    bf16 = mybir.dt.bfloat16
    f32 = mybir.dt.float32
    KT = Hd // P       # GEMM1 contraction tiles (over H)
    IT = Id // P       # i-tiles (GEMM1 output partitions / GEMM2 contraction)
    TT = C // nb       # token blocks for GEMM1 moving operand
    T8 = C // P        # token tiles for GEMM2 output partitions
    HB = Hd // nb      # h-blocks for GEMM2 moving operand
    IP = IT // 2       # i-block pairs
    W2 = 2 * P         # pair width in I columns

    nc = bacc.Bacc(
        "TRN2",
        target_bir_lowering=False,
        debug=False,
        enable_asserts=False,
        num_devices=8,
    )
    xT = nc.dram_tensor("xT", [Hd, C], bf16, kind="ExternalInput").ap()
    wg = nc.dram_tensor("wg", [Hd, Id], bf16, kind="ExternalInput").ap()
    wu = nc.dram_tensor("wu", [Hd, Id], bf16, kind="ExternalInput").ap()
    wd = nc.dram_tensor("wd", [Id, Hd], bf16, kind="ExternalInput").ap()
    out = nc.dram_tensor("out", [C, Hd], bf16, kind="ExternalOutput").ap()

    def wpair(w, p):
        """Native [Hd, Id] cols [p*256, (p+1)*256) gathered as [hp, kt, c]."""
        return w[:, p * W2:(p + 1) * W2].rearrange("(k p) c -> p k c", p=P)

    def wpair_kchunk(w, p, k0, k1):
        """Rows kt in [k0, k1) of the pair gather (startup chunking)."""
        return w[k0 * P:k1 * P, p * W2:(p + 1) * W2].rearrange(
            "(k p) c -> p k c", p=P)

    def sb_kview(tl, k0, k1):
        return tl[:, k0 * W2:k1 * W2].rearrange("p (k c) -> p k c", k=k1 - k0)

    def wdblock_kchunk(h, k0, k1):
        """Native [Id, Hd] cols [h*nb, (h+1)*nb), rows [k0*P, k1*P) gathered
        as [ip, ki, hh]."""
        return wd[k0 * P:k1 * P, h * nb:(h + 1) * nb].rearrange(
            "(k p) h -> p k h", p=P)

    with tile.TileContext(nc) as tc:
        with tc.tile_pool(name="res", bufs=1) as res:
            # SwiGLU intermediate, transposed: interT[p, i*C + c] = inter[c, i*P+p]
            interT = res.tile([P, IT * C], bf16)
            # h=0 block of Wd, prefetched during phase 1 so phase 2 starts hot
            wd0 = res.tile([P, IT * nb], bf16)

            # ps1 spans BOTH phases (6 banks: g-tags x2, u-tags x1) so the
            # allocator must give ps2 disjoint fresh banks — otherwise the
            # first phase-2 matmul inherits a conservative wait on ALL
            # phase-1 matmuls completing (bank-reuse WAW gate, ~0.87us).
            with tc.tile_pool(name="ps1", bufs=2, space="PSUM") as ps1:
              # ------------- Phase 1: gate/up GEMMs + SwiGLU -------------
              with tc.tile_pool(name="p1x", bufs=1) as p1x, \
                 tc.tile_pool(name="w1", bufs=3) as w1, \
                 tc.tile_pool(name="tmp1", bufs=4) as tmp1:
                  # Startup: HWDGE issue is ~0.6us per dma_start per ring, so
                  # the ramp is bound by issue slots, not HBM bandwidth.  xt
                  # loads as KT large chunks arriving in the order the first
                  # psum group consumes them; the first weight pair and the
                  # first xt chunk are further split so the first real matmul
                  # fires as early as possible.
                  wgp0 = w1.tile([P, KT * W2], bf16, tag="wg")
                  wup0 = w1.tile([P, KT * W2], bf16, tag="wu")
                  xt = p1x.tile([P, KT * C], bf16)
                  # xt[p, k*C + c] = x[c, k*P+p]  (resident, 32KB/partition).
                  # One DMA per k-block: Tile's completion semaphores are
                  # per-instruction, so coarser transfers stall the first
                  # matmul group on the whole-DMA completion (measured +7us).
                  # Pair 0 loads its j=0 column half first — the j=1 half is
                  # not consumed until i=1 (~28us in), and the ramp window is
                  # HBM-oversubscribed, so deferring it lands xt sooner.
                  wgv = wg[:, 0:W2].rearrange("(k p) c -> p k c", p=P)
                  wuv = wu[:, 0:W2].rearrange("(k p) c -> p k c", p=P)
                  g0v = wgp0[:, :].rearrange("p (k c) -> p k c", c=W2)
                  u0v = wup0[:, :].rearrange("p (k c) -> p k c", c=W2)
                  KH = KT // 2
                  for k in range(KT):
                    eng = nc.sync if k % 2 == 0 else nc.scalar
                    eng.dma_start(xt[:, k * C:(k + 1) * C], xT[k * P:(k + 1) * P, :])
                    if k == 0:
                        nc.sync.dma_start(g0v[:, 0:KH, 0:P], wgv[:, 0:KH, 0:P])
                        nc.scalar.dma_start(u0v[:, 0:KH, 0:P], wuv[:, 0:KH, 0:P])
                    elif k == 2:
                        nc.sync.dma_start(g0v[:, KH:KT, 0:P], wgv[:, KH:KT, 0:P])
                        nc.scalar.dma_start(u0v[:, KH:KT, 0:P], wuv[:, KH:KT, 0:P])
                  # j=1 columns of pair 0, after the ramp-critical stream
                  nc.sync.dma_start(g0v[:, :, P:W2], wgv[:, :, P:W2])
                  nc.scalar.dma_start(u0v[:, :, P:W2], wuv[:, :, P:W2])
                  for p in range(IP):
                    if p == 0:
                        wgp, wup = wgp0, wup0
                    else:
                        wgp = w1.tile([P, KT * W2], bf16, tag="wg")
                        nc.sync.dma_start(sb_kview(wgp, 0, KT), wpair(wg, p))
                        wup = w1.tile([P, KT * W2], bf16, tag="wu")
                        nc.scalar.dma_start(sb_kview(wup, 0, KT), wpair(wu, p))
                        if p == 4:
                            # prefetch Wd h=0 once the startup ramp has
                            # drained; phase 2 needs it at ~2/3 of the span
                            for d in range(4):
                                kk = IT // 4
                                eng = nc.sync if d % 2 == 0 else nc.scalar
                                eng.dma_start(
                                    wd0[:, d * kk * nb:(d + 1) * kk * nb]
                                    .rearrange("p (k h) -> p k h", k=kk),
                                    wdblock_kchunk(0, d * kk, (d + 1) * kk))
                    for j in range(2):
                        i = 2 * p + j
                        for t in range(TT):
                            psg = ps1.tile([P, nb], f32, tag=f"g{t}")
                            psu = ps1.tile([P, nb], f32, tag=f"u{t}", bufs=1)
                            for k in range(KT):
                                rhs = xt[:, k * C + t * nb: k * C + t * nb + nb]
                                lhs = wgp[:, k * W2 + j * P: k * W2 + (j + 1) * P]
                                nc.tensor.matmul(psg[:], lhs, rhs,
                                                 start=(k == 0), stop=(k == KT - 1))
                            for k in range(KT):
                                rhs = xt[:, k * C + t * nb: k * C + t * nb + nb]
                                lhs = wup[:, k * W2 + j * P: k * W2 + (j + 1) * P]
                                nc.tensor.matmul(psu[:], lhs, rhs,
                                                 start=(k == 0), stop=(k == KT - 1))
                            # silu(g)*u = sigmoid(g)*g*u; each DVE op may
                            # read at most ONE operand from PSUM.
                            sig = tmp1.tile([P, nb], f32, tag="sig")
                            nc.scalar.activation(
                                sig[:], psg[:], mybir.ActivationFunctionType.Sigmoid)
                            sg = tmp1.tile([P, nb], f32, tag="sg")
                            nc.vector.tensor_mul(sg[:], sig[:], psg[:])
                            nc.vector.tensor_mul(
                                interT[:, i * C + t * nb: i * C + t * nb + nb],
                                sg[:], psu[:])

              # ---------------- Phase 2: down GEMM ----------------
              with tc.tile_pool(name="w2", bufs=2) as w2, \
                 tc.tile_pool(name="ps2", bufs=2, space="PSUM") as ps2, \
                 tc.tile_pool(name="ot2", bufs=4) as ot2:
                  for h in range(HB):
                    if h == 0:
                        wdh = wd0
                    else:
                        wdh = w2.tile([P, IT * nb], bf16, tag="wd")
                        for d in range(2):
                            kk = IT // 2
                            eng = nc.sync if d % 2 == 0 else nc.scalar
                            eng.dma_start(
                                wdh[:, d * kk * nb:(d + 1) * kk * nb]
                                .rearrange("p (k h) -> p k h", k=kk),
                                wdblock_kchunk(h, d * kk, (d + 1) * kk))
                    for t in range(T8):
                        ps = ps2.tile([P, nb], f32, tag="o")
                        for k in range(IT):
                            nc.tensor.matmul(
                                ps[:],
                                interT[:, k * C + t * P: k * C + t * P + P],
                                wdh[:, k * nb:(k + 1) * nb],
                                start=(k == 0), stop=(k == IT - 1))
                        ot = ot2.tile([P, nb], bf16, tag="ot")
                        nc.scalar.copy(ot[:], ps[:])
                        nc.sync.dma_start(out[t * P:(t + 1) * P, h * nb:(h + 1) * nb], ot[:])

    nc.compile()
    return nc


def _get_nc(C, Hd, Id):
    key = (C, Hd, Id)
    if key not in _NC_CACHE:
        _NC_CACHE[key] = _build(C, Hd, Id)
    return _NC_CACHE[key]


def _prepare(inputs):
    """Host-side dispatch: split tokens by expert, pad to capacity, cast to
    bf16.  Weights keep their native layouts — the device program's
    rearranged-AP DMAs do the tiling, so host prep is cast-only."""
    bf = ml_dtypes.bfloat16
    x = np.asarray(inputs["permuted_local_hidden_states"], dtype=np.float32)
    tpe = np.asarray(inputs["tokens_per_expert"], dtype=np.int64)
    gate = np.asarray(inputs["gate_proj"], dtype=np.float32)
    up = np.asarray(inputs["up_proj"], dtype=np.float32)
    down = np.asarray(inputs["down_proj"], dtype=np.float32)

    Ee, Hd, Id = gate.shape
    Tt = x.shape[0]
    assert Ee == E, f"expected {E} experts, got {Ee}"
    counts = [int(c) for c in tpe]
    starts = [0]
    for c in counts:
        starts.append(starts[-1] + c)
    cmax = max(max(counts), 1)
    # round capacity to a multiple of NB so TT = C//NB tiles exactly
    C = max(C_DEFAULT, ((cmax + NB - 1) // NB) * NB)

    in_maps = []
    for e in range(Ee):
        s, cnt = starts[e], counts[e]
        if cnt == C:
            xe = x[s:s + cnt]
        else:
            xe = np.zeros((C, Hd), np.float32)
            xe[:cnt] = x[s:s + cnt]
        in_maps.append({
            "xT": np.ascontiguousarray(xe.T).astype(bf),
            "wg": gate[e].astype(bf),
            "wu": up[e].astype(bf),
            "wd": down[e].astype(bf),
        })
    meta = (Tt, Hd, starts, counts, C)
    return in_maps, meta


def _postprocess(results, meta):
    Tt, Hd, starts, counts, _C = meta
    outf = np.zeros((Tt, Hd), np.float32)
    for e in range(len(counts)):
        s, cnt = starts[e], counts[e]
        if cnt > 0:
            outf[s:s + cnt] = np.asarray(results[e]["out"])[:cnt].astype(np.float32)
    return outf


def kernel(**inputs):
    from concourse.bass_utils import run_bass_kernel_spmd
    in_maps, meta = _prepare(inputs)
    nc = _get_nc(meta[4], meta[1], np.asarray(inputs["gate_proj"]).shape[2])
    res = run_bass_kernel_spmd(nc, in_maps, list(range(E)))
    return _postprocess(res.results, meta)

